# revision 13
# baseline (speedup 1.0000x reference)
"""Fused transformer block (LN -> MHA -> LN -> FFN) on 8 TRN2 NeuronCores, v3.

Same sharding contract as v1/v2 (core c: batch c//2, token half c%2, K/V
duplicated within the pair, no collectives).

v3 = v2's fp8 DoubleRow math + aggressive cross-phase interleaving:
- K projection (pairs 1..7) and the heads-8..15 V chunk are emitted inside
  the qc=0 attention sweep, riding tensor-engine slack while ACT chews exps.
- out-proj + LN2 + fc1 + fc2 for tokens 0..511 are emitted inside the qc=1
  attention sweep; only the second token half remains as tail work.
- rstd = Rsqrt(var+eps) in one ACT op; fc1 outputs stage through a bf16
  buffer so gelu runs as 4 batched 2048-column ACT ops per half instead of
  64 interleaved ones (avoids LoadActFuncSet ping-pong with Exp).
- kt is stored fp8 (mixed fp8-stationary x bf16-moving scores matmul) to fit
  the SBUF budget.
- PSUM: A phase pj(4 banks)+sm(2); attention sc(4)+cps(2)+ctp(1)+misc(1)=8,
  where misc doubles as the psum for interleaved kproj/vproj/out-proj/fc.
"""

from contextlib import ExitStack

import ml_dtypes
import numpy as np

import concourse.bass as bass
import concourse.mybir as mybir
import concourse.tile as tile
from concourse import bacc
from concourse.masks import make_identity

F32 = mybir.dt.float32
BF16 = mybir.dt.bfloat16
FP8 = mybir.dt.float8e4
AF = mybir.ActivationFunctionType
ALU = mybir.AluOpType
DR = mybir.MatmulPerfMode.DoubleRow

B_FULL = 4
S_FULL = 2048
D_FULL = 1024
H_FULL = 16
FF_FULL = 2048
HD = 64
EPS = 1e-5
N_CORES = 8
P = 128

LAST_EXEC_NS = None
LAST_RESULTS = None
LAST_NC = None


def build_nc(S=S_FULL, T=S_FULL // 2, D=D_FULL, H=H_FULL, FF=FF_FULL):
    assert H * HD == D
    DT = D // P
    TT_ALL = S // P
    TT_OWN = T // P
    FT = FF // P
    HPD = P // HD
    GS = min(512, D)
    NG = D // GS
    QC = 512
    ESC = float(HD) ** -0.5 / 4096.0

    nc = bacc.Bacc("TRN2", target_bir_lowering=False, debug=False,
                   enable_asserts=False, num_devices=N_CORES)

    xb_d = nc.dram_tensor("xb", [S, D], BF16, kind="ExternalInput").ap()
    xb2_d = nc.dram_tensor("xb2", [T, D], BF16, kind="ExternalInput").ap()
    wq_d = nc.dram_tensor("wq", [D, D], FP8, kind="ExternalInput").ap()
    wk_d = nc.dram_tensor("wk", [D, D], FP8, kind="ExternalInput").ap()
    wv_d = nc.dram_tensor("wv", [D, D], FP8, kind="ExternalInput").ap()
    wo_d = nc.dram_tensor("wo", [D, D], FP8, kind="ExternalInput").ap()
    w1_d = nc.dram_tensor("w1", [D, FF], FP8, kind="ExternalInput").ap()
    w2_d = nc.dram_tensor("w2", [FF, D], FP8, kind="ExternalInput").ap()
    out_d = nc.dram_tensor("out", [T, D], F32, kind="ExternalOutput").ap()

    with tile.TileContext(nc) as tc:
      with ExitStack() as stack:
        small = stack.enter_context(tc.tile_pool(name="small", bufs=1))
        ident = small.tile([P, P], BF16, name="ident")
        make_identity(nc, ident)
        eps_t = small.tile([P, 1], F32, name="eps_t")
        nc.vector.memset(eps_t, EPS)

        # ---- persistent SBUF (bottom of stack, longest-lived first) ----
        p_x2 = tc.alloc_tile_pool(name="p_x2", bufs=1)
        x2 = p_x2.tile([P, TT_OWN, D], BF16, name="x2")
        p_ctxt = tc.alloc_tile_pool(name="p_ctxt", bufs=1)
        ctxt = p_ctxt.tile([P, DT, T], FP8, name="ctxt")
        p_xn2t = tc.alloc_tile_pool(name="p_xn2t", bufs=1)
        xn2t = p_xn2t.tile([P, DT, TT_OWN, P], FP8, name="xn2t")
        p_qt = tc.alloc_tile_pool(name="p_qt", bufs=1)
        qt = p_qt.tile([P, DT, T], BF16, name="qt")
        p_kt = tc.alloc_tile_pool(name="p_kt", bufs=1)
        kt = p_kt.tile([P, DT, S], FP8, name="kt")
        p_va = tc.alloc_tile_pool(name="p_va", bufs=1)
        v_aug = p_va.tile([P, TT_ALL, H, HD + 1], FP8, name="v_aug")
        nc.vector.memset(v_aug[:, :, :, HD:HD + 1], 0.5)
        p_xb2 = tc.alloc_tile_pool(name="p_xb2", bufs=1)
        xb2_sb = p_xb2.tile([P, TT_OWN, D], BF16, name="xb2_sb")
        exp_pool = tc.alloc_tile_pool(name="exp_pool", bufs=5)
        ctx_pool = tc.alloc_tile_pool(name="ctx_pool", bufs=4)

        # ---- phase A pools: xnt + wk live into B0; wq/wv/x die at A end ----
        p_xnt = tc.alloc_tile_pool(name="p_xnt", bufs=1)
        xnt = p_xnt.tile([P, DT, TT_ALL, P], FP8, name="xnt")
        p_wk = tc.alloc_tile_pool(name="p_wk", bufs=1)
        wk_sb = p_wk.tile([P, DT, D], FP8, name="wk_sb")
        p_wv = tc.alloc_tile_pool(name="p_wv", bufs=1)
        wv_sb = p_wv.tile([P, DT, D], FP8, name="wv_sb")
        p_wq = tc.alloc_tile_pool(name="p_wq", bufs=1)
        wq_sb = p_wq.tile([P, DT, D], FP8, name="wq_sb")
        p_xs = tc.alloc_tile_pool(name="p_xs", bufs=6)
        ln_pool = tc.alloc_tile_pool(name="ln_pool", bufs=4)

        pj_pool = tc.alloc_tile_pool(name="pj_pool", bufs=2, space="PSUM")
        sm_pool = tc.alloc_tile_pool(name="sm_pool", bufs=2, space="PSUM")

        # ---------------- phase A: LN1 -> xnt; Q; kproj p0; vproj c0 ----------
        for tt in range(TT_ALL):
            x_t = p_xs.tile([P, D], BF16, tag="xs", name="x_t")
            nc.sync.dma_start(out=x_t, in_=xb_d[P * tt:P * (tt + 1), :])
            stats = ln_pool.tile([P, NG, 6], F32, tag="st", name="stats")
            for g in range(NG):
                nc.vector.bn_stats(out=stats[:, g, :],
                                   in_=x_t[:, GS * g:GS * (g + 1)])
            mv = ln_pool.tile([P, 2], F32, tag="mv", name="mv")
            nc.vector.bn_aggr(out=mv, in_=stats)
            std = ln_pool.tile([P, 1], F32, tag="sd", name="std")
            nc.scalar.activation(out=std, in_=mv[:, 1:2], func=AF.Sqrt,
                                 bias=eps_t, scale=1.0)
            rstd = ln_pool.tile([P, 1], F32, tag="rs", name="rstd")
            nc.vector.reciprocal(out=rstd, in_=std)
            xn_t = ln_pool.tile([P, D], BF16, tag="xn", name="xn_t")
            nc.gpsimd.tensor_scalar(out=xn_t, in0=x_t, scalar1=mv[:, 0:1],
                                    scalar2=rstd, op0=ALU.subtract, op1=ALU.mult)
            for dt0 in range(0, DT, 4):
                tp = sm_pool.tile([P, 4 * P], BF16, tag="sm", name="tp")
                for j in range(4):
                    nc.tensor.transpose(tp[:, P * j:P * (j + 1)],
                                        xn_t[:, P * (dt0 + j):P * (dt0 + j + 1)],
                                        ident)
                nc.scalar.activation(out=xnt[:, dt0:dt0 + 4, tt, :], in_=tp,
                                     func=AF.Identity, scale=1.0)

        for dt in range(DT):
            nc.sync.dma_start(out=wq_sb[:, dt, :], in_=wq_d[P * dt:P * (dt + 1), :])
        for dt in range(DT):
            nc.sync.dma_start(out=wk_sb[:, dt, :], in_=wk_d[P * dt:P * (dt + 1), :])
        for dt in range(DT):
            nc.sync.dma_start(out=wv_sb[:, dt, :], in_=wv_d[P * dt:P * (dt + 1), :])

        def emit_qproj(db):
            dot, po = db // HPD, HD * (db % HPD)
            pp = pj_pool.tile([HD, 2, QC], F32, tag="pj", name="ppq")
            for c in range(2):
                for j in range(DT // 2):
                    nc.tensor.matmul(
                        pp[:, c, :],
                        wq_sb[:, 2 * j:2 * j + 2, P * dot + po:P * dot + po + HD],
                        xnt[:, 2 * j:2 * j + 2, (QC // P) * c:(QC // P) * (c + 1), :],
                        start=(j == 0), stop=(j == DT // 2 - 1), perf_mode=DR)
            if db % 2 == 0:
                nc.vector.tensor_copy(out=qt[po:po + HD, dot, :], in_=pp)
            else:
                nc.scalar.activation(out=qt[po:po + HD, dot, :], in_=pp,
                                     func=AF.Identity, scale=1.0)

        def mk_ps(pool, tag):
            t = pool.tile([HD, 2, QC] if tag == "pj" else [HD, QC],
                          F32, tag=tag, name="ps_" + tag)
            return (t[:, 0, :] if tag == "pj" else t)

        def emit_kproj(pr, pool, tag):
            for jj in range(2):
                po = HD * jj
                db = 2 * pr + jj
                for kc in range(S // QC):
                    pk = mk_ps(pool, tag)
                    for j in range(DT // 2):
                        nc.tensor.matmul(
                            pk,
                            wk_sb[:, 2 * j:2 * j + 2, HD * db:HD * (db + 1)],
                            xnt[:, 2 * j:2 * j + 2,
                                (QC // P) * kc:(QC // P) * (kc + 1), :],
                            start=(j == 0), stop=(j == DT // 2 - 1), perf_mode=DR)
                    nc.vector.tensor_copy(
                        out=kt[po:po + HD, pr, QC * kc:QC * (kc + 1)], in_=pk)

        def emit_vproj(c, tts, pool, tag, eng_alt):
            for tt in tts:
                for j in range(2):
                    pv = mk_ps(pool, tag)
                    for k in range(DT // 2):
                        nc.tensor.matmul(
                            pv,
                            xnt[:, 2 * k:2 * k + 2, tt, HD * j:HD * (j + 1)],
                            wv_sb[:, 2 * k:2 * k + 2, QC * c:QC * (c + 1)],
                            start=(k == 0), stop=(k == DT // 2 - 1), perf_mode=DR)
                    if eng_alt:
                        nc.scalar.activation(
                            out=v_aug[HD * j:HD * (j + 1), tt,
                                      8 * c:8 * (c + 1), 0:HD],
                            in_=pv, func=AF.Identity, scale=1.0 / 64.0)
                    else:
                        nc.vector.tensor_scalar(
                            out=v_aug[HD * j:HD * (j + 1), tt,
                                      8 * c:8 * (c + 1), 0:HD],
                            in0=pv, scalar1=1.0 / 64.0, scalar2=None,
                            op0=ALU.mult)

        for db in range(D // HD):
            emit_qproj(db)
        emit_kproj(0, pj_pool, "pj")
        emit_vproj(0, range(TT_ALL), pj_pool, "pj", True)
        ln_pool.release()
        p_xs.release()
        p_wq.release()
        sm_pool.release()
        pj_pool.release()

        # ---------------- attention + interleaved work ----------------
        sc_pool = tc.alloc_tile_pool(name="sc_pool", bufs=2, space="PSUM")
        cps_pool = tc.alloc_tile_pool(name="cps_pool", bufs=2, space="PSUM")
        ctp_pool = tc.alloc_tile_pool(name="ctp_pool", bufs=1, space="PSUM")
        misc_pool = tc.alloc_tile_pool(name="misc_pool", bufs=1, space="PSUM")

        def emit_scores(h, qc):
            pr, po = h // HPD, HD * (h % HPD)
            halves = []
            for hf in range(2):
                expt = exp_pool.tile([P, TT_ALL // 2, QC], FP8, tag="expt",
                                     name="expt")
                for sp in range(TT_ALL // 4):
                    ring = sc_pool.tile([P, 2, QC], F32, tag="sc", name="sc")
                    for jj in range(2):
                        st = hf * (TT_ALL // 2) + 2 * sp + jj
                        nc.tensor.matmul(
                            ring[:, jj, :],
                            kt[po:po + HD, pr, P * st:P * (st + 1)],
                            qt[po:po + HD, pr, QC * qc:QC * (qc + 1)],
                            start=True, stop=True)
                    nc.scalar.activation(out=expt[:, 2 * sp:2 * sp + 2, :],
                                         in_=ring, func=AF.Exp, scale=ESC)
                halves.append(expt)
            return halves

        def emit_ctx(h, qc, halves):
            pr, po = h // HPD, HD * (h % HPD)
            for cg in range(2):
                cps = cps_pool.tile([HD, 4, HD + 1], F32, tag="cps", name="cps")
                for qi in range(4):
                    qb = 4 * cg + qi
                    for sp in range(TT_ALL // 2):
                        expt = halves[sp // (TT_ALL // 4)]
                        s4 = sp % (TT_ALL // 4)
                        nc.tensor.matmul(
                            cps[:, qi, :],
                            expt[:, 2 * s4:2 * s4 + 2, HD * qb:HD * (qb + 1)],
                            v_aug[:, 2 * sp:2 * sp + 2, h, :],
                            start=(sp == 0), stop=(sp == TT_ALL // 2 - 1),
                            perf_mode=DR)
                rec = ctx_pool.tile([HD, 4], F32, tag="rec", name="rec")
                den_ap = bass.AP(tensor=cps.tensor, offset=cps.offset + HD,
                                 ap=[[cps.ap[0][0], HD], [HD + 1, 4]])
                nc.vector.reciprocal(out=rec, in_=den_ap)
                csb = ctx_pool.tile([HD, 4, HD], BF16, tag="csb", name="csb")
                num_ap = bass.AP(tensor=cps.tensor, offset=cps.offset,
                                 ap=[[cps.ap[0][0], HD], [HD + 1, 4], [1, HD]])
                rec_b = bass.AP(tensor=rec.tensor, offset=rec.offset,
                                ap=[[rec.ap[0][0], HD], [1, 4], [0, HD]])
                nc.vector.tensor_tensor(out=csb, in0=num_ap, in1=rec_b,
                                        op=ALU.mult)
                ctp = ctp_pool.tile([P, 4 * P], BF16, tag="ctp", name="ctp")
                for qi in range(4):
                    nc.tensor.transpose(ctp[0:HD, HD * qi:HD * (qi + 1)],
                                        csb[:, qi, :], ident[0:HD, 0:HD])
                nc.vector.tensor_copy(
                    out=ctxt[po:po + HD, pr,
                             QC * qc + 4 * HD * cg:QC * qc + 4 * HD * (cg + 1)],
                    in_=ctp[0:HD, 0:4 * HD])

        def emit_outproj_tile(tt):
            for j in range(2):
                for c in range(2):
                    pp = misc_pool.tile([HD, QC], F32, tag="ms", name="ppo")
                    for k in range(DT // 2):
                        nc.tensor.matmul(
                            pp,
                            ctxt[:, 2 * k:2 * k + 2,
                                 P * tt + HD * j:P * tt + HD * (j + 1)],
                            wo_sb[:, 2 * k:2 * k + 2, QC * c:QC * (c + 1)],
                            start=(k == 0), stop=(k == DT // 2 - 1), perf_mode=DR)
                    nc.vector.scalar_tensor_tensor(
                        out=x2[HD * j:HD * (j + 1), tt, QC * c:QC * (c + 1)],
                        in0=pp, scalar=1.0 / 1024.0,
                        in1=xb2_sb[HD * j:HD * (j + 1), tt, QC * c:QC * (c + 1)],
                        op0=ALU.mult, op1=ALU.add)

        def emit_ln2_stats(tt, pool):
            stats = pool.tile([P, NG, 6], F32, tag="st2", name="stats2")
            for g in range(NG):
                nc.vector.bn_stats(out=stats[:, g, :],
                                   in_=x2[:, tt, GS * g:GS * (g + 1)])
            mv = pool.tile([P, 2], F32, tag="mv2", name="mv2")
            nc.vector.bn_aggr(out=mv, in_=stats)
            return mv

        def emit_ln2_norm(tt, mv, rstd, pool, on_act=False):
            xn_t = pool.tile([P, D], BF16, tag="xn2", name="xn2_t")
            nc.gpsimd.tensor_scalar(out=xn_t, in0=x2[:, tt, :],
                                    scalar1=mv[:, 0:1], scalar2=rstd,
                                    op0=ALU.subtract, op1=ALU.mult)
            for dt0 in range(0, DT, 4):
                tp = ctp_pool.tile([P, 4 * P], BF16, tag="ctp", name="tp2")
                for j in range(4):
                    nc.tensor.transpose(tp[:, P * j:P * (j + 1)],
                                        xn_t[:, P * (dt0 + j):P * (dt0 + j + 1)],
                                        ident)
                if on_act:
                    nc.scalar.activation(out=xn2t[:, dt0:dt0 + 4, tt, :],
                                         in_=tp, func=AF.Identity, scale=1.0)
                else:
                    nc.vector.tensor_copy(out=xn2t[:, dt0:dt0 + 4, tt, :],
                                          in_=tp)

        def emit_fc1_unit(ft, j, tc4, on_act=False):
            pf = misc_pool.tile([HD, QC], F32, tag="ms", name="pf")
            for k in range(DT // 2):
                nc.tensor.matmul(
                    pf,
                    w1_sb[:, 2 * k:2 * k + 2, P * ft + HD * j:P * ft + HD * (j + 1)],
                    xn2t[:, 2 * k:2 * k + 2, tc4:tc4 + 4, :],
                    start=(k == 0), stop=(k == DT // 2 - 1), perf_mode=DR)
            if on_act and ft % 2 == 0:
                nc.scalar.activation(out=htp[HD * j:HD * (j + 1), ft, :],
                                     in_=pf, func=AF.Identity, scale=1.0 / 64.0)
            else:
                nc.vector.tensor_scalar(
                    out=htp[HD * j:HD * (j + 1), ft, :], in0=pf,
                    scalar1=1.0 / 64.0, scalar2=None, op0=ALU.mult)

        def emit_gelu_batch(tc4):
            for g in range(FT // 4):
                nc.scalar.activation(
                    out=ht[:, 4 * g:4 * (g + 1), P * tc4:P * (tc4 + 4)],
                    in_=htp[:, 4 * g:4 * (g + 1), :], func=AF.Gelu, scale=1.0)

        def emit_fc2_unit(tt, j, c):
            pp = misc_pool.tile([HD, QC], F32, tag="ms", name="ppf2")
            for k in range(FT // 2):
                nc.tensor.matmul(
                    pp,
                    ht[:, 2 * k:2 * k + 2, P * tt + HD * j:P * tt + HD * (j + 1)],
                    w2_sb[:, 2 * k:2 * k + 2, QC * c:QC * (c + 1)],
                    start=(k == 0), stop=(k == FT // 2 - 1), perf_mode=DR)
            o_sb = out_pool.tile([HD, QC], F32, tag="osb", name="o_sb")
            nc.vector.scalar_tensor_tensor(
                out=o_sb, in0=pp, scalar=1.0 / 64.0,
                in1=x2[HD * j:HD * (j + 1), tt, QC * c:QC * (c + 1)],
                op0=ALU.mult, op1=ALU.add)
            nc.sync.dma_start(
                out=out_d[P * tt + HD * j:P * tt + HD * (j + 1),
                          QC * c:QC * (c + 1)], in_=o_sb)

        # --- qc = 0 sweep: attention + kproj pairs 1..7 + vproj c=1 ---
        prev = None
        for h in range(H):
            halves = emit_scores(h, 0)
            if prev is not None:
                emit_ctx(*prev)
            prev = (h, 0, halves)
            if h % 2 == 0 and 1 + h // 2 < DT:
                emit_kproj(1 + h // 2, misc_pool, "ms")
            if h == 1:
                emit_vproj(1, range(0, TT_ALL // 2), misc_pool, "ms", False)
            if h == 3:
                emit_vproj(1, range(TT_ALL // 2, TT_ALL), misc_pool, "ms", False)
                p_wv.release()
            if h == 15:
                p_wk.release()
                p_xnt.release()

        # late pools (allocated after xnt/wk die; released at the very end)
        p_ht = tc.alloc_tile_pool(name="p_ht", bufs=1)
        ht = p_ht.tile([P, FT, T], FP8, name="ht")
        p_htp = tc.alloc_tile_pool(name="p_htp", bufs=1)
        htp = p_htp.tile([P, FT, QC], BF16, name="htp")
        p_w1 = tc.alloc_tile_pool(name="p_w1", bufs=1)
        w1_sb = p_w1.tile([P, DT, FF], FP8, name="w1_sb")
        for dt in range(DT):
            nc.sync.dma_start(out=w1_sb[:, dt, :], in_=w1_d[P * dt:P * (dt + 1), :])
        p_w2 = tc.alloc_tile_pool(name="p_w2", bufs=1)
        w2_sb = p_w2.tile([P, FT, D], FP8, name="w2_sb")
        for ft in range(FT):
            nc.sync.dma_start(out=w2_sb[:, ft, :], in_=w2_d[P * ft:P * (ft + 1), :])
        p_wo = tc.alloc_tile_pool(name="p_wo", bufs=1)
        wo_sb = p_wo.tile([P, DT, D], FP8, name="wo_sb")
        for dt in range(DT):
            nc.sync.dma_start(out=wo_sb[:, dt, :], in_=wo_d[P * dt:P * (dt + 1), :])
        ln2_pool = tc.alloc_tile_pool(name="ln2_pool", bufs=4)
        out_pool = tc.alloc_tile_pool(name="out_pool", bufs=4)
        for tt in range(TT_OWN):
            nc.sync.dma_start(out=xb2_sb[:, tt, :],
                              in_=xb2_d[P * tt:P * (tt + 1), :])

        # --- qc = 1 sweep: attention + C/D for tokens 0..QC-1 (tt 0..3) ---
        mv_list = [None] * TT_OWN
        for h in range(H):
            halves = emit_scores(h, 1)
            emit_ctx(*prev)
            prev = (h, 1, halves)
            if h < 4:
                emit_outproj_tile(h)
            elif h == 4:
                for tt in range(4):
                    mv_list[tt] = emit_ln2_stats(tt, ln2_pool)
            elif h == 5:
                std4 = ln2_pool.tile([P, 4], F32, tag="sd2", name="std4")
                for tt in range(4):
                    nc.scalar.activation(out=std4[:, tt:tt + 1],
                                         in_=mv_list[tt][:, 1:2],
                                         func=AF.Sqrt, bias=eps_t, scale=1.0)
                rstd4 = ln2_pool.tile([P, 4], F32, tag="rs2", name="rstd4")
                nc.vector.reciprocal(out=rstd4, in_=std4)
                for tt in range(4):
                    emit_ln2_norm(tt, mv_list[tt], rstd4[:, tt:tt + 1], ln2_pool)
            elif 6 <= h <= 13:
                base = 4 * (h - 6)
                for u in range(4):
                    ft, j = divmod(base + u, 2)
                    emit_fc1_unit(ft, j, 0)
                if h == 13:
                    emit_gelu_batch(0)
            elif h == 14:
                for u in range(8):
                    tt, jc = divmod(u, 2)
                    emit_fc2_unit(tt, jc, 0)
            elif h == 15:
                for u in range(8):
                    tt, jc = divmod(u, 2)
                    emit_fc2_unit(tt, jc, 1)
        emit_ctx(*prev)

        # ---------------- tail: tokens QC..T-1 (tt 4..7) ----------------
        # swap the attention psum pools for deep-buffered tail pools so the
        # out-proj/fc chains pipeline instead of lockstepping with DVE copies
        misc_pool.release()
        ctp_pool.release()
        cps_pool.release()
        sc_pool.release()
        misc_pool = tc.alloc_tile_pool(name="tail_ms", bufs=6, space="PSUM")
        ctp_pool = tc.alloc_tile_pool(name="tail_tp", bufs=2, space="PSUM")
        for tt in range(4, TT_OWN):
            emit_outproj_tile(tt)
        for tt in range(4, TT_OWN):
            mv_list[tt] = emit_ln2_stats(tt, ln2_pool)
        std4b = ln2_pool.tile([P, 4], F32, tag="sd2", name="std4b")
        for tt in range(4, TT_OWN):
            nc.scalar.activation(out=std4b[:, tt - 4:tt - 3],
                                 in_=mv_list[tt][:, 1:2],
                                 func=AF.Sqrt, bias=eps_t, scale=1.0)
        rstd4b = ln2_pool.tile([P, 4], F32, tag="rs2", name="rstd4b")
        nc.vector.reciprocal(out=rstd4b, in_=std4b)
        for tt in range(4, TT_OWN):
            emit_ln2_norm(tt, mv_list[tt], rstd4b[:, tt - 4:tt - 3], ln2_pool,
                          on_act=(tt % 2 == 0))
        for ft in range(FT):
            for j in range(2):
                emit_fc1_unit(ft, j, 4, on_act=True)
        emit_gelu_batch(4)
        for tt in range(4, TT_OWN):
            for j in range(2):
                for c in range(2):
                    emit_fc2_unit(tt, j, c)

        out_pool.release()
        ln2_pool.release()
        p_wo.release()
        p_w2.release()
        p_w1.release()
        p_htp.release()
        p_ht.release()
        ctp_pool.release()
        misc_pool.release()
        ctx_pool.release()
        exp_pool.release()
        p_xb2.release()
        p_va.release()
        p_kt.release()
        p_qt.release()
        p_xn2t.release()
        p_ctxt.release()
        p_x2.release()
    nc.compile()
    return nc


def _fold_host(inputs):
    f = {k: np.asarray(v, dtype=np.float32) for k, v in inputs.items()}
    g1, b1, g2, b2 = f["g1"], f["b1"], f["g2"], f["b2"]
    for bias, w in (("bq", "Wq"), ("bk", "Wk"), ("bv", "Wv")):
        assert np.abs(b1 @ f[w] + f[bias]).max() == 0.0
    assert np.abs(f["bo"]).max() == 0.0
    assert np.abs(b2 @ f["W1"] + f["bf1"]).max() == 0.0
    assert np.abs(f["bf2"]).max() == 0.0
    f8 = lambda a: np.ascontiguousarray(a).astype(ml_dtypes.float8_e4m3)
    w = {
        "wq": f8(64.0 * g1[:, None] * f["Wq"]),
        "wk": f8(64.0 * g1[:, None] * f["Wk"]),
        "wv": f8(512.0 * g1[:, None] * f["Wv"]),
        "wo": f8(64.0 * f["Wo"]),
        "w1": f8(64.0 * g2[:, None] * f["W1"]),
        "w2": f8(64.0 * f["W2"]),
    }
    return f, w


def kernel(**inputs):
    global LAST_EXEC_NS, LAST_RESULTS, LAST_NC
    import os

    from concourse.bass_utils import run_bass_kernel_spmd

    f, w = _fold_host(inputs)
    x = f["x"]
    B, S, D = x.shape
    T = S // 2
    nc = build_nc(S=S, T=T, D=D, H=H_FULL, FF=FF_FULL)
    LAST_NC = nc

    in_maps = []
    for c in range(N_CORES):
        b, half = c // 2, c % 2
        if half == 0:
            xc = x[b]
        else:
            xc = np.concatenate([x[b, T:], x[b, :T]], axis=0)
        m = {"xb": np.ascontiguousarray(xc).astype(ml_dtypes.bfloat16),
             "xb2": np.ascontiguousarray(xc[:T] + f["bo"]).astype(
                 ml_dtypes.bfloat16)}
        m.update(w)
        in_maps.append(m)

    trace = bool(int(os.environ.get("KBENCH_TRACE", "0")))
    res = run_bass_kernel_spmd(nc, in_maps, list(range(N_CORES)), trace=trace)
    LAST_EXEC_NS = res.exec_time_ns
    LAST_RESULTS = res

    out = np.empty((B, S, D), dtype=np.float32)
    for c in range(N_CORES):
        b, half = c // 2, c % 2
        out[b, T * half:T * (half + 1)] = res.results[c]["out"]
    return out
